# revision 1
# baseline (speedup 1.0000x reference)
"""BitNet Transformer MLP on 8 Trainium2 NeuronCores.

Math (per reference):
  sw1 = max(mean|W1|, EPS); wq1 = clip(round(W1/sw1), -1, 1)
  sx[t] = max(max_h|x[t,h]|, EPS)/127; xq = round(x/sx)      (ints in [-127,127])
  h = gelu((xq @ wq1.T) * sx * sw1)                           (exact erf gelu)
  sh[t] = max(max_i|h[t,i]|, EPS)/127; hq = round(h/sh)
  out = (hq @ wq2.T) * sh * sw2

Sharding (tensor-parallel over the intermediate dim I):
  - tokens T flattened; core c quantizes its T/8 token slice (in transposed
    layout), AllGather -> xqT (bf16, exact), chunk-interleaved by rank
  - core c holds W1 rows [c*I/8:(c+1)*I/8] and W2 cols [same I-slice];
    the host feeds the shards PRE-TRANSPOSED (pure layout, no math) so every
    device-side load is contiguous: w1t=[H, I/8], w2t=[I/8, H], xt=[H, T/8]
  - per-tensor weight scales via a 2-float AllReduce of |W| partial sums
  - fc1 computes the h.T slice [I/8, T] locally (PE contracts H on partitions)
  - per-token max|h| partials -> one AllReduce(max) of [T]
  - fc2 computes partial out.T [H, T]; ReduceScatter(add) per token-block
    gives core c the final out.T rows [c*H/8:(c+1)*H/8]
  - host concatenates the 8 H-slices and transposes back.

All matmuls run in bf16, which is EXACT here: quantized activations are
integers <=127 and weights are ternary, both exactly representable in
bf16; accumulation is fp32 in PSUM. The intermediate h is spilled in bf16.
"""

import numpy as np

import concourse.bass as bass
import concourse.mybir as mybir
import concourse.tile as tile
from concourse import bass_utils, bacc

F32 = mybir.dt.float32
BF16 = mybir.dt.bfloat16
MAGIC = 12582912.0  # 1.5*2^23: (v+MAGIC)-MAGIC == round-to-nearest-even, |v|<2^22
EPS = 1e-5
Alu = mybir.AluOpType
Act = mybir.ActivationFunctionType

# full problem config
B, S, H, I = 4, 2048, 4096, 16384
T = B * S
NCORES = 8


def build_program(T=T, H=H, I=I, ncores=NCORES, nb=512, w1_halves=2,
                  stop_after=None, repeat=1, h_bf16=True):
    """Build the SPMD program (same on all cores). Returns compiled Bacc."""
    TS = T // ncores          # token shard (quant phase)
    IS = I // ncores          # I shard per core
    HS = H // ncores          # H shard of the final output per core
    NBLK = T // nb            # token blocks
    KH = H // 128             # contraction tiles for fc1
    KI = IS // 128            # contraction tiles for fc2
    IH = IS // w1_halves      # fc1 weight-resident half size
    MIH = IH // 128           # fc1 m-tiles per half
    MH = H // 128             # fc2 m-tiles
    NT32 = nb // 32           # 32-token groups per block
    CHUNK = 512               # phase-Q free-dim chunk
    BPC = TS // nb            # token blocks per AG rank-chunk
    HDT = mybir.dt.float16 if h_bf16 else F32
    assert nb % 128 == 0 and TS % 128 == 0 and IS % 128 == 0 and TS % nb == 0
    assert KI % 4 == 0 or KI < 4

    nc = bacc.Bacc("TRN2", target_bir_lowering=False, debug=False, num_devices=ncores)

    x_e = nc.dram_tensor("x", [TS, H], F32, kind="ExternalInput")
    xt_e = nc.dram_tensor("xt", [H, TS], F32, kind="ExternalInput")
    w1t_e = nc.dram_tensor("w1t", [H, IS], F32, kind="ExternalInput")
    w2t_e = nc.dram_tensor("w2t", [IS, H], F32, kind="ExternalInput")
    out_e = nc.dram_tensor("out_t", [HS, T], F32, kind="ExternalOutput")

    rg = [list(range(ncores))]

    with tile.TileContext(nc) as tc:
        with (
            tc.tile_pool(name="singles", bufs=1) as singles,
            tc.tile_pool(name="work", bufs=3) as work,
            tc.tile_pool(name="bigw", bufs=1) as bigw,
            tc.tile_pool(name="xqp", bufs=2) as xqp,
            tc.tile_pool(name="hqp", bufs=1) as hqp,
            tc.tile_pool(name="stage", bufs=2) as stage,
            tc.tile_pool(name="outp", bufs=2) as outp,
            tc.tile_pool(name="psum", bufs=4, space="PSUM") as psum,
            tc.tile_pool(name="psbc", bufs=2, space="PSUM") as psbc,
            tc.tile_pool(name="dram", bufs=1, space="DRAM") as dram,
        ):
            # ---------------- DRAM scratch ----------------
            xq_ag_in = dram.tile([H, TS], BF16, name="xq_ag_in")
            # chunk-interleaved: rank c's tokens live at rows [c*H:(c+1)*H]
            xqT_full = dram.tile([ncores * H, TS], BF16, name="xqT_full", addr_space="Shared")
            sx_ag_in = dram.tile([TS], F32, name="sx_ag_in")
            sx_full = dram.tile([T], F32, name="sx_full", addr_space="Shared")
            wsum_in = dram.tile([1, 2], F32, name="wsum_in")
            wsum_out = dram.tile([1, 2], F32, name="wsum_out", addr_space="Shared")
            w1q = dram.tile([H, IS], BF16, name="w1q")
            w2q = dram.tile([IS, H], BF16, name="w2q")
            h_dram = dram.tile([IS, T], HDT, name="h_dram")
            hmax_in = dram.tile([T], F32, name="hmax_in")
            hmax_outs = [dram.tile([T], F32, name=f"hmax_out_{r}", addr_space="Shared")
                         for r in range(max(repeat, 1))]
            rs_in = [dram.tile([H, nb], mybir.dt.float16, name=f"rs_in_{j}") for j in range(NBLK)]
            rs_out = [dram.tile([HS, nb], mybir.dt.float16, name=f"rs_out_{j}") for j in range(NBLK)]

            # ---------------- constants ----------------
            ones_row = singles.tile([1, 128], F32, name="ones_row")
            nc.any.memset(ones_row[:], 1.0)
            ones_col = singles.tile([128, 1], F32, name="ones_col")
            nc.any.memset(ones_col[:], 1.0)

            def bcast_row(row_ap, n, tag="bc"):
                """[1, n] SBUF row -> [128, n] tile (PE ones outer product)."""
                ps = psbc.tile([128, nb], F32, tag="psbc")
                nc.tensor.matmul(ps[:, :n], lhsT=ones_row[:], rhs=row_ap, start=True, stop=True)
                t = stage.tile([128, nb], F32, tag=tag)
                nc.vector.tensor_copy(t[:, :n], ps[:, :n])
                return t

            # ---------------- phase Q: x scales (token shard, natural layout) ----------------
            NXT = TS // 128
            for it in range(NXT):
                rowmax = stage.tile([128, 1], F32, tag="rmax")
                for ch, c0 in enumerate(range(0, H, CHUNK)):
                    cw = min(CHUNK, H - c0)
                    xtile = work.tile([128, CHUNK], F32, tag="cf32")
                    nc.sync.dma_start(
                        xtile[:, :cw], x_e[it * 128:(it + 1) * 128, c0:c0 + cw]
                    )
                    part = stage.tile([128, 1], F32, tag="part")
                    nc.vector.tensor_reduce(part[:], xtile[:, :cw], axis=mybir.AxisListType.X,
                                            op=Alu.max, apply_absolute_value=True)
                    if ch == 0:
                        nc.vector.tensor_copy(rowmax[:], part[:])
                    else:
                        nc.vector.tensor_tensor(rowmax[:], rowmax[:], part[:], Alu.max)
                nc.vector.tensor_scalar_max(rowmax[:], rowmax[:], EPS)
                nc.sync.dma_start(
                    sx_ag_in[it * 128:(it + 1) * 128].rearrange("(p a) -> p a", a=1),
                    rowmax[:],
                )

            nc.gpsimd.collective_compute(
                "AllGather", Alu.bypass, replica_groups=rg,
                ins=[sx_ag_in[:].opt()], outs=[sx_full[:].opt()],
            )

            # quantize x in transposed layout, per token-column group
            for c0 in range(0, TS, CHUNK):
                cw = min(CHUNK, TS - c0)
                rq_row = stage.tile([1, nb], F32, tag="srow")
                nc.sync.dma_start(rq_row[:, :cw],
                                  sx_ag_in[c0:c0 + cw].rearrange("(a f) -> a f", a=1))
                nc.vector.reciprocal(rq_row[:, :cw], rq_row[:, :cw])
                nc.vector.tensor_scalar_mul(rq_row[:, :cw], rq_row[:, :cw], 127.0)
                ps = psbc.tile([128, nb], F32, tag="psbc")
                nc.tensor.matmul(ps[:, :cw], lhsT=ones_row[:], rhs=rq_row[:, :cw],
                                 start=True, stop=True)
                rq_bcc = stage.tile([128, nb], F32, tag="bc")
                nc.vector.tensor_copy(rq_bcc[:, :cw], ps[:, :cw])
                for it in range(H // 128):
                    xtile = work.tile([128, CHUNK], F32, tag="cf32")
                    nc.sync.dma_start(xtile[:, :cw], xt_e[it * 128:(it + 1) * 128, c0:c0 + cw])
                    nc.vector.tensor_tensor(xtile[:, :cw], xtile[:, :cw], rq_bcc[:, :cw], Alu.mult)
                    xqt = work.tile([128, CHUNK], BF16, tag="cbf")
                    nc.vector.tensor_scalar(xqt[:, :cw], xtile[:, :cw], MAGIC, MAGIC, Alu.add, Alu.subtract)
                    nc.sync.dma_start(xq_ag_in[it * 128:(it + 1) * 128, c0:c0 + cw], xqt[:, :cw])

            nc.gpsimd.collective_compute(
                "AllGather", Alu.bypass, replica_groups=rg,
                ins=[xq_ag_in[:].opt()], outs=[xqT_full[:].opt()],
            )

            # ---------------- phase Q: weight |.| sums ----------------
            skip_w = stop_after == "xq"

            def abs_sum_partial(src, rows, cols, tag):
                acc = singles.tile([128, 1], F32, name=f"acc_{tag}")
                first = True
                for it in range(rows // 128):
                    for c0 in range(0, cols, CHUNK):
                        cw = min(CHUNK, cols - c0)
                        wt = work.tile([128, CHUNK], F32, tag="cf32")
                        nc.sync.dma_start(
                            wt[:, :cw], src[it * 128:(it + 1) * 128, c0:c0 + cw]
                        )
                        part = stage.tile([128, 1], F32, tag="part")
                        nc.vector.tensor_reduce(part[:], wt[:, :cw], axis=mybir.AxisListType.X,
                                                op=Alu.add, apply_absolute_value=True)
                        if first:
                            nc.vector.tensor_copy(acc[:], part[:])
                            first = False
                        else:
                            nc.vector.tensor_tensor(acc[:], acc[:], part[:], Alu.add)
                return acc

            if not skip_w:
                acc1 = abs_sum_partial(w1t_e, H, IS, "w1")
                acc2 = abs_sum_partial(w2t_e, IS, H, "w2")
                wsum_sb = singles.tile([1, 2], F32, name="wsum_sb")
                for idx, acc in ((0, acc1), (1, acc2)):
                    pss_full = psbc.tile([128, nb], F32, tag="psbc")
                    pss = pss_full[0:1, 0:1]
                    nc.tensor.matmul(pss, lhsT=acc[:], rhs=ones_col[:], start=True, stop=True)
                    nc.vector.tensor_copy(wsum_sb[0:1, idx:idx + 1], pss)
                nc.sync.dma_start(wsum_in[:, :], wsum_sb[:])
                nc.gpsimd.collective_compute(
                    "AllReduce", Alu.add, replica_groups=rg,
                    ins=[wsum_in[:].opt()], outs=[wsum_out[:].opt()],
                )
                sw_sb = singles.tile([1, 2], F32, name="sw_sb")
                nc.sync.dma_start(sw_sb[:], wsum_out[:, :])
                nc.vector.tensor_scalar_mul(sw_sb[:], sw_sb[:], 1.0 / (I * H))
                nc.vector.tensor_scalar_max(sw_sb[:], sw_sb[:], EPS)
                rsw_sb = singles.tile([1, 2], F32, name="rsw_sb")
                nc.vector.reciprocal(rsw_sb[:], sw_sb[:])

                def bcast_scalar(src_ap, name):
                    ps_full = psbc.tile([128, nb], F32, tag="psbc")
                    ps = ps_full[:, 0:1]
                    nc.tensor.matmul(ps, lhsT=ones_row[:], rhs=src_ap, start=True, stop=True)
                    t = singles.tile([128, 1], F32, name=name)
                    nc.vector.tensor_copy(t[:], ps)
                    return t

                rsw1_col = bcast_scalar(rsw_sb[0:1, 0:1], "rsw1_col")
                rsw2_col = bcast_scalar(rsw_sb[0:1, 1:2], "rsw2_col")
                sw1_127_col = bcast_scalar(sw_sb[0:1, 0:1], "sw1_127_col")
                nc.vector.tensor_scalar_mul(sw1_127_col[:], sw1_127_col[:], 1.0 / 127.0)
                sw2_127_col = bcast_scalar(sw_sb[0:1, 1:2], "sw2_127_col")
                nc.vector.tensor_scalar_mul(sw2_127_col[:], sw2_127_col[:], 1.0 / 127.0)

                def quantize_weights(src, dst, rows, cols, rsw_col):
                    for it in range(rows // 128):
                        for c0 in range(0, cols, CHUNK):
                            cw = min(CHUNK, cols - c0)
                            wt = work.tile([128, CHUNK], F32, tag="cf32")
                            nc.sync.dma_start(
                                wt[:, :cw], src[it * 128:(it + 1) * 128, c0:c0 + cw]
                            )
                            nc.scalar.mul(wt[:, :cw], wt[:, :cw], rsw_col[:])
                            nc.vector.tensor_scalar(wt[:, :cw], wt[:, :cw], MAGIC, MAGIC,
                                                    Alu.add, Alu.subtract)
                            wq = work.tile([128, CHUNK], BF16, tag="cbf")
                            nc.vector.tensor_scalar(wq[:, :cw], wt[:, :cw], 1.0, -1.0,
                                                    Alu.min, Alu.max)
                            nc.sync.dma_start(
                                dst[it * 128:(it + 1) * 128, c0:c0 + cw], wq[:, :cw]
                            )

                quantize_weights(w1t_e, w1q, H, IS, rsw1_col)
                quantize_weights(w2t_e, w2q, IS, H, rsw2_col)

            # ---------------- fc1 ----------------
            for _rep in range(repeat if stop_after not in ("q", "xq") else 0):
                hred_acc = [singles.tile([32, NT32], F32, name=f"hred_{j}_{_rep}")
                            for j in range(NBLK)]
                for half in range(w1_halves):
                    w1qT = bigw.tile([128, KH, IH], BF16, tag="bigw")
                    for k in range(KH):
                        nc.sync.dma_start(
                            w1qT[:, k, :],
                            w1q[k * 128:(k + 1) * 128, half * IH:(half + 1) * IH],
                        )
                    for blk in range(NBLK):
                        crk = blk // BPC           # AG rank chunk
                        coff = (blk % BPC) * nb    # token offset within chunk
                        xq_sb = xqp.tile([128, KH, nb], BF16, tag="xq")
                        for k in range(KH):
                            nc.sync.dma_start(
                                xq_sb[:, k, :],
                                xqT_full[crk * H + k * 128: crk * H + (k + 1) * 128,
                                         coff:coff + nb],
                            )
                        s_row = stage.tile([1, nb], F32, tag="srow")
                        nc.sync.dma_start(
                            s_row[:],
                            sx_full[blk * nb:(blk + 1) * nb].rearrange("(a f) -> a f", a=1),
                        )
                        m1_t = bcast_row(s_row[:], nb)
                        nc.vector.tensor_scalar(m1_t[:], m1_t[:], sw1_127_col[:], None, Alu.mult)

                        gmax = stage.tile([128, nb], HDT, tag="gmax")
                        for m in range(MIH):
                            ps = psum.tile([128, nb], F32, tag="ps1")
                            for k in range(KH):
                                nc.tensor.matmul(
                                    ps[:], lhsT=w1qT[:, k, m * 128:(m + 1) * 128],
                                    rhs=xq_sb[:, k, :],
                                    start=(k == 0), stop=(k == KH - 1),
                                )
                            g = work.tile([128, nb], F32, tag="g")
                            nc.vector.tensor_tensor(g[:], ps[:], m1_t[:], Alu.mult)
                            gq = work.tile([128, nb], HDT, tag="gq")
                            nc.scalar.activation(gq[:], g[:], Act.Gelu)
                            nc.sync.dma_start(
                                h_dram[half * IH + m * 128: half * IH + (m + 1) * 128,
                                       blk * nb:(blk + 1) * nb],
                                gq[:],
                            )
                            gabs = work.tile([128, nb], HDT, tag="tmph")
                            nc.scalar.activation(gabs[:], gq[:], Act.Abs)
                            if m == 0:
                                nc.vector.tensor_copy(gmax[:], gabs[:])
                            else:
                                nc.vector.tensor_tensor(gmax[:], gmax[:], gabs[:], Alu.max)
                        ftmp = stage.tile([64, nb], HDT, tag="foldt")
                        nc.vector.tensor_copy(ftmp[0:64], gmax[64:128])
                        nc.vector.tensor_tensor(gmax[0:64], gmax[0:64], ftmp[0:64], Alu.max)
                        nc.vector.tensor_copy(ftmp[0:32], gmax[32:64])
                        nc.vector.tensor_tensor(gmax[0:32], gmax[0:32], ftmp[0:32], Alu.max)
                        gmt = stage.tile([32, nb], HDT, tag="foldt")
                        nc.vector.transpose(gmt[:], gmax[0:32, :])
                        red = stage.tile([32, NT32], F32, tag="red")
                        nc.vector.tensor_reduce(
                            red[:], gmt[:].rearrange("p (c q) -> p c q", q=32),
                            axis=mybir.AxisListType.X, op=Alu.max,
                        )
                        if half == 0:
                            nc.vector.tensor_copy(hred_acc[blk][:], red[:])
                        else:
                            nc.vector.tensor_tensor(hred_acc[blk][:], hred_acc[blk][:],
                                                    red[:], Alu.max)

                for blk in range(NBLK):
                    nc.sync.dma_start(
                        hmax_in[blk * nb:(blk + 1) * nb].rearrange("(c p) -> p c", p=32),
                        hred_acc[blk][:],
                    )
                nc.gpsimd.collective_compute(
                    "AllReduce", Alu.max, replica_groups=rg,
                    ins=[hmax_in[:].opt()], outs=[hmax_outs[_rep][:].opt()],
                )

            # ---------------- fc2 ----------------
            for _rep in range(repeat if stop_after not in ("q", "xq", "fc1") else 0):
                if KI >= 4:
                    splits = [(0, KI // 2, "bigw", bigw),
                              (KI // 2, (3 * KI) // 4, "xq", xqp),
                              ((3 * KI) // 4, KI, "xq", xqp)]
                else:
                    splits = [(0, KI, "bigw", bigw)]
                w2_tiles = []
                for (k0, k1, tag, pool) in splits:
                    wt2 = pool.tile([128, k1 - k0, H], BF16, tag=tag)
                    for ki in range(k0, k1):
                        nc.sync.dma_start(
                            wt2[:, ki - k0, :], w2q[ki * 128:(ki + 1) * 128, :]
                        )
                    w2_tiles.append((k0, k1, wt2))

                def w2_lhsT(ki, msl):
                    for (k0, k1, wt2) in w2_tiles:
                        if k0 <= ki < k1:
                            return wt2[:, ki - k0, msl]
                    raise AssertionError

                for blk in range(NBLK):
                    s_row = stage.tile([1, nb], F32, tag="srow")
                    nc.sync.dma_start(
                        s_row[:],
                        hmax_outs[_rep][blk * nb:(blk + 1) * nb].rearrange("(a f) -> a f", a=1),
                    )
                    nc.vector.tensor_scalar_max(s_row[:], s_row[:], EPS)
                    r_row = stage.tile([1, nb], F32, tag="srow")
                    nc.vector.reciprocal(r_row[:], s_row[:])
                    rq_t = bcast_row(r_row[:], nb)
                    nc.vector.tensor_scalar_mul(rq_t[:], rq_t[:], 127.0)
                    m2_t = bcast_row(s_row[:], nb)
                    nc.vector.tensor_scalar(m2_t[:], m2_t[:], sw2_127_col[:], None, Alu.mult)

                    hq = hqp.tile([128, KI, nb], BF16, tag="hq")
                    for ki in range(KI):
                        ht = work.tile([128, nb], F32, tag="tmpf")
                        hin = work.tile([128, nb], HDT, tag="gq")
                        nc.sync.dma_start(
                            hin[:], h_dram[ki * 128:(ki + 1) * 128, blk * nb:(blk + 1) * nb]
                        )
                        nc.vector.tensor_tensor(ht[:], hin[:], rq_t[:], Alu.mult)
                        nc.vector.tensor_scalar(hq[:, ki, :], ht[:], MAGIC, MAGIC,
                                                Alu.add, Alu.subtract)
                    for m in range(MH):
                        ps = psum.tile([128, nb], F32, tag="ps1")
                        msl = slice(m * 128, (m + 1) * 128)
                        for ki in range(KI):
                            nc.tensor.matmul(
                                ps[:], lhsT=w2_lhsT(ki, msl), rhs=hq[:, ki, :],
                                start=(ki == 0), stop=(ki == KI - 1),
                            )
                        ot = outp.tile([128, nb], mybir.dt.float16, tag="ot")
                        nc.scalar.copy(ot[:], ps[:])
                        nc.sync.dma_start(rs_in[blk][m * 128:(m + 1) * 128, :], ot[:])
                    nc.gpsimd.collective_compute(
                        "ReduceScatter", Alu.add, replica_groups=rg,
                        ins=[rs_in[blk][:].opt()], outs=[rs_out[blk][:].opt()],
                    )
                    # post-RS per-token scaling on the owned H-slice
                    for q0 in range(0, HS, 128):
                        qw = min(128, HS - q0)
                        rt = outp.tile([128, nb], mybir.dt.float16, tag="rt")
                        nc.sync.dma_start(
                            rt[:qw], rs_out[blk][q0:q0 + qw, :]
                        )
                        of = outp.tile([128, nb], F32, tag="of")
                        nc.vector.tensor_tensor(of[:qw], rt[:qw], m2_t[:qw], Alu.mult)
                        nc.sync.dma_start(
                            out_e[q0:q0 + qw, blk * nb:(blk + 1) * nb], of[:qw]
                        )

    nc.compile()
    return nc


_PROGRAM_CACHE = {}


def _get_program(key):
    if key not in _PROGRAM_CACHE:
        _PROGRAM_CACHE[key] = build_program(*key)
    return _PROGRAM_CACHE[key]


def make_in_maps(x, W1, W2, ncores=NCORES):
    t, h = x.reshape(-1, x.shape[-1]).shape
    i = W1.shape[0]
    xf = np.ascontiguousarray(x.reshape(t, h), dtype=np.float32)
    ts, isd = t // ncores, i // ncores
    in_maps = []
    for c in range(ncores):
        xs = xf[c * ts:(c + 1) * ts]
        in_maps.append({
            "x": xs,
            "xt": np.ascontiguousarray(xs.T),
            "w1t": np.ascontiguousarray(W1[c * isd:(c + 1) * isd, :].T, dtype=np.float32),
            "w2t": np.ascontiguousarray(W2[:, c * isd:(c + 1) * isd].T, dtype=np.float32),
        })
    return in_maps


def run(x, W1, W2, trace=False, trace_kwargs=None):
    """Run the distributed kernel on full inputs. Returns (out, BassKernelResults)."""
    t, h = x.reshape(-1, x.shape[-1]).shape
    i = W1.shape[0]
    nc = _get_program((t, h, i, NCORES))
    in_maps = make_in_maps(x, W1, W2)
    res = bass_utils.run_bass_kernel_spmd(
        nc, in_maps, core_ids=list(range(NCORES)), trace=trace,
        **(trace_kwargs or {}),
    )
    out_t = np.concatenate([res.results[c]["out_t"] for c in range(NCORES)], axis=0)
    out = np.ascontiguousarray(out_t.T).reshape(x.shape)
    return out, res


def kernel(x, W1, W2):
    out, _ = run(x, W1, W2)
    return out


class TimedRunner:
    """Compile once, keep inputs on device, time repeated executions.

    Mirrors bass2jax.run_bass_via_pjrt's multi-core path but persists the
    device-side inputs so repeat calls measure (dispatch + HW execution)
    only, not the host->device staging.
    """

    def __init__(self, nc, in_maps):
        import jax
        import concourse.mybir as mybir_
        from concourse import bass2jax
        from jax.experimental.shard_map import shard_map
        from jax.sharding import Mesh, PartitionSpec, NamedSharding

        bass2jax.install_neuronx_cc_hook()
        n_cores = len(in_maps)
        partition_name = nc.partition_id_tensor.name if nc.partition_id_tensor else None
        in_names, out_names, out_avals = [], [], []
        for alloc in nc.m.functions[0].allocations:
            if not isinstance(alloc, mybir_.MemoryLocationSet):
                continue
            name = alloc.memorylocations[0].name
            if alloc.kind == "ExternalInput":
                if name != partition_name:
                    in_names.append(name)
            elif alloc.kind == "ExternalOutput":
                out_names.append(name)
                out_avals.append(jax.core.ShapedArray(
                    tuple(alloc.tensor_shape), mybir_.dt.np(alloc.dtype)))
        n_params = len(in_names)
        n_outs = len(out_avals)
        all_in_names = list(in_names) + list(out_names)
        if partition_name is not None:
            all_in_names.append(partition_name)
        donate = tuple(range(n_params, n_params + n_outs))

        def _body(*args):
            operands = list(args)
            if partition_name is not None:
                operands.append(bass2jax.partition_id_tensor())
            outs = bass2jax._bass_exec_p.bind(
                *operands,
                out_avals=tuple(out_avals),
                in_names=tuple(all_in_names),
                out_names=tuple(out_names),
                lowering_input_output_aliases=(),
                sim_require_finite=True,
                sim_require_nnan=True,
                nc=nc,
            )
            return tuple(outs)

        devices = jax.devices()[:n_cores]
        mesh = Mesh(np.asarray(devices), ("core",))
        in_specs = (PartitionSpec("core"),) * (n_params + n_outs)
        out_specs = (PartitionSpec("core"),) * n_outs
        self._fn = jax.jit(
            shard_map(_body, mesh=mesh, in_specs=in_specs, out_specs=out_specs,
                      check_rep=False),
            donate_argnums=donate, keep_unused=True,
        )
        sh = NamedSharding(mesh, PartitionSpec("core"))
        concat_in = [
            np.concatenate([np.asarray(in_maps[c][nm]) for c in range(n_cores)], axis=0)
            for nm in in_names
        ]
        self._dev_in = [jax.device_put(a, sh) for a in concat_in]
        self._zero_shapes = [(n_cores * a.shape[0], *a.shape[1:]) for a in out_avals]
        self._zero_dtypes = [a.dtype for a in out_avals]
        self._sh = sh
        self._jax = jax
        self.out_names = out_names
        self.out_avals = out_avals
        self.n_cores = n_cores

    def run_once(self):
        import time
        jax = self._jax
        zeros = [jax.device_put(np.zeros(s, d), self._sh)
                 for s, d in zip(self._zero_shapes, self._zero_dtypes)]
        jax.block_until_ready(zeros)
        t0 = time.perf_counter()
        outs = self._fn(*self._dev_in, *zeros)
        jax.block_until_ready(outs)
        dt = time.perf_counter() - t0
        results = [
            {nm: np.asarray(outs[i]).reshape(self.n_cores, *self.out_avals[i].shape)[c]
             for i, nm in enumerate(self.out_names)}
            for c in range(self.n_cores)
        ]
        return results, dt



# revision 6
# speedup vs baseline: 18.2116x; 18.2116x over previous
"""BitNet Transformer MLP on 8 Trainium2 NeuronCores.

Math (per reference):
  sw1 = max(mean|W1|, EPS); wq1 = clip(round(W1/sw1), -1, 1)
  sx[t] = max(max_h|x[t,h]|, EPS)/127; xq = round(x/sx)      (ints in [-127,127])
  h = gelu((xq @ wq1.T) * sx * sw1)                           (exact erf gelu)
  sh[t] = max(max_i|h[t,i]|, EPS)/127; hq = round(h/sh)
  out = (hq @ wq2.T) * sh * sw2

Sharding (tensor-parallel over the intermediate dim I):
  - tokens T flattened; core c quantizes its T/8 token slice in transposed
    layout, AllGather -> xqT (bf16, exact), chunk-interleaved by rank
  - core c holds W1 rows [c*I/8:(c+1)*I/8] and W2 cols [same I-slice];
    the host feeds the shards PRE-TRANSPOSED (pure layout, no math):
    w1t=[H, I/8], w2t=[I/8, H], xt=[H, T/8]
  - per-tensor weight scales via a 2-float AllReduce of |W| partial sums,
    issued FIRST so the AllReduce hides under the x-quantization phase
  - weights are quantized on the fly while loading into SBUF for fc1/fc2
    (no DRAM round-trip for quantized weights)
  - fc1 computes the h.T slice [I/8, T] locally (PE contracts H on partitions)
  - per-token max|h| partials -> one AllReduce(max) of [T]
  - fc2 computes partial out.T [H, T] PRE-SCALED by sh*sw2/127 so the
    ReduceScatter(add) directly produces final f16 outputs; core c gets
    out.T rows [c*H/8:(c+1)*H/8], copied DRAM->DRAM to the output
  - host concatenates the 8 H-slices, upcasts f16->f32 and transposes back.

All matmuls run in bf16, which is EXACT here: quantized activations are
integers <=127 and weights are ternary, both exactly representable in
bf16; accumulation is fp32 in PSUM. The intermediate h is spilled in fp16.
"""

import numpy as np

import concourse.bass as bass
import concourse.mybir as mybir
import concourse.tile as tile
from concourse import bass_utils, bacc

F32 = mybir.dt.float32
F16 = mybir.dt.float16
BF16 = mybir.dt.bfloat16
MAGIC = 12582912.0  # 1.5*2^23: (v+MAGIC)-MAGIC == round-to-nearest-even, |v|<2^22
EPS = 1e-5
Alu = mybir.AluOpType
Act = mybir.ActivationFunctionType

# full problem config
B, S, H, I = 4, 2048, 4096, 16384
T = B * S
NCORES = 8


def build_program(T=T, H=H, I=I, ncores=NCORES, nb=512, w1_halves=2,
                  rs_blocks=4, outer_repeat=1, stop_after=None):
    """Build the SPMD program (same on all cores). Returns compiled Bacc."""
    TS = T // ncores          # token shard (quant phase)
    IS = I // ncores          # I shard per core
    HS = H // ncores          # H shard of the final output per core
    NBLK = T // nb            # token blocks
    KH = H // 128             # contraction tiles for fc1
    KI = IS // 128            # contraction tiles for fc2
    IH = IS // w1_halves      # fc1 weight-resident half size
    MIH = IH // 128           # fc1 m-tiles per half
    MH = H // 128             # fc2 m-tiles
    NT32 = nb // 32           # 32-token groups per block
    CHUNK = 512               # quant-phase free-dim chunk
    BPC = TS // nb            # token blocks per AG rank-chunk
    NG = NBLK // rs_blocks    # ReduceScatter groups
    GT = rs_blocks * nb       # tokens per RS group
    assert nb % 128 == 0 and TS % 128 == 0 and IS % 128 == 0 and TS % nb == 0
    assert NBLK % rs_blocks == 0

    nc = bacc.Bacc("TRN2", target_bir_lowering=False, debug=False, num_devices=ncores)

    xt_e = nc.dram_tensor("xt", [H, TS], F32, kind="ExternalInput")
    w1t_e = nc.dram_tensor("w1t", [H, IS], F32, kind="ExternalInput")
    w2t_e = nc.dram_tensor("w2t", [IS, H], F32, kind="ExternalInput")
    out_e = nc.dram_tensor("out_t", [HS, T], F16, kind="ExternalOutput")

    rg = [list(range(ncores))]

    with tile.TileContext(nc) as tc:
        with (
            tc.tile_pool(name="singles", bufs=1) as singles,
            tc.tile_pool(name="work", bufs=3) as work,
            tc.tile_pool(name="bigw", bufs=1) as bigw,
            tc.tile_pool(name="xqp", bufs=2) as xqp,
            tc.tile_pool(name="hqp", bufs=2) as hqp,
            tc.tile_pool(name="stage", bufs=2) as stage,
            tc.tile_pool(name="outp", bufs=2) as outp,
            tc.tile_pool(name="psum", bufs=4, space="PSUM") as psum,
            tc.tile_pool(name="psbc", bufs=2, space="PSUM") as psbc,
            tc.tile_pool(name="dram", bufs=1, space="DRAM") as dram,
        ):
            # ---------------- DRAM scratch ----------------
            # collective OUTPUT buffers must be single-writer: per-rep copies
            xq_ag_in = dram.tile([H, TS], BF16, name="xq_ag_in")
            sx_ag_in = dram.tile([TS], F32, name="sx_ag_in")
            wsum_in = dram.tile([1, 2], F32, name="wsum_in")
            h_dram = dram.tile([IS, T], F16, name="h_dram")
            hmax_in = dram.tile([T], F32, name="hmax_in")
            rs_in = [dram.tile([H, GT], F16, name=f"rs_in_{g}") for g in range(NG)]
            R = outer_repeat
            # chunk-interleaved: rank c's tokens live at rows [c*H:(c+1)*H]
            xqT_fulls = [dram.tile([ncores * H, TS], BF16, name=f"xqT_full_{r}",
                                   addr_space="Shared") for r in range(R)]
            sx_fulls = [dram.tile([T], F32, name=f"sx_full_{r}", addr_space="Shared")
                        for r in range(R)]
            wsum_outs = [dram.tile([1, 2], F32, name=f"wsum_out_{r}", addr_space="Shared")
                         for r in range(R)]
            hmax_outs = [dram.tile([T], F32, name=f"hmax_out_{r}", addr_space="Shared")
                         for r in range(R)]
            rs_outs = [[dram.tile([HS, GT], F16, name=f"rs_out_{g}_{r}")
                        for g in range(NG)] for r in range(R)]

            # ---------------- constants ----------------
            ones_row = singles.tile([1, 128], F32, name="ones_row")
            nc.any.memset(ones_row[:], 1.0)
            ones_col = singles.tile([128, 1], F32, name="ones_col")
            nc.any.memset(ones_col[:], 1.0)

            def bcast_row(row_ap, n, tag="bc", bufs=None):
                """[1, n] SBUF row -> [128, n] tile (PE ones outer product)."""
                ps = psbc.tile([128, nb], F32, tag="psbc")
                nc.tensor.matmul(ps[:, :n], lhsT=ones_row[:], rhs=row_ap,
                                 start=True, stop=True)
                t = stage.tile([128, nb], F32, tag=tag, bufs=bufs)
                nc.vector.tensor_copy(t[:, :n], ps[:, :n])
                return t

            def emit_body(rep):
                xqT_full = xqT_fulls[rep]
                sx_full = sx_fulls[rep]
                wsum_out = wsum_outs[rep]
                hmax_out = hmax_outs[rep]
                rs_out = rs_outs[rep]
                # ---------- phase W-abs: |W| partial sums -> AllReduce ----------
                def abs_sum_partial(src, rows, cols, tag):
                    acc = singles.tile([128, 1], F32, name=f"acc_{tag}")
                    first = True
                    for it in range(rows // 128):
                        for c0 in range(0, cols, CHUNK):
                            cw = min(CHUNK, cols - c0)
                            wt = work.tile([128, CHUNK], F32, tag="cf32")
                            nc.sync.dma_start(
                                wt[:, :cw], src[it * 128:(it + 1) * 128, c0:c0 + cw]
                            )
                            part = stage.tile([128, 1], F32, tag="part")
                            nc.vector.tensor_reduce(
                                part[:], wt[:, :cw], axis=mybir.AxisListType.X,
                                op=Alu.add, apply_absolute_value=True)
                            if first:
                                nc.vector.tensor_copy(acc[:], part[:])
                                first = False
                            else:
                                nc.vector.tensor_tensor(acc[:], acc[:], part[:], Alu.add)
                    return acc

                acc1 = abs_sum_partial(w1t_e, H, IS, "w1")
                acc2 = abs_sum_partial(w2t_e, IS, H, "w2")
                wsum_sb = singles.tile([1, 2], F32, name="wsum_sb")
                for idx, acc in ((0, acc1), (1, acc2)):
                    pss_full = psbc.tile([128, nb], F32, tag="psbc")
                    pss = pss_full[0:1, 0:1]
                    nc.tensor.matmul(pss, lhsT=acc[:], rhs=ones_col[:],
                                     start=True, stop=True)
                    nc.vector.tensor_copy(wsum_sb[0:1, idx:idx + 1], pss)
                nc.sync.dma_start(wsum_in[:, :], wsum_sb[:])
                nc.gpsimd.collective_compute(
                    "AllReduce", Alu.add, replica_groups=rg,
                    ins=[wsum_in[:].opt()], outs=[wsum_out[:].opt()],
                )

                # ---------- phase Q1: per-token max|x| from transposed x ------
                for ci, c0 in enumerate(range(0, TS, CHUNK)):
                    cw = min(CHUNK, TS - c0)
                    amax = stage.tile([128, CHUNK], F32, tag="amax", bufs=1)
                    amin = stage.tile([128, CHUNK], F32, tag="amin", bufs=1)
                    for it in range(KH):
                        xtile = work.tile([128, CHUNK], F32, tag="cf32")
                        nc.sync.dma_start(
                            xtile[:, :cw], xt_e[it * 128:(it + 1) * 128, c0:c0 + cw]
                        )
                        if it == 0:
                            nc.vector.tensor_copy(amax[:, :cw], xtile[:, :cw])
                            nc.vector.tensor_copy(amin[:, :cw], xtile[:, :cw])
                        else:
                            nc.vector.tensor_tensor(amax[:, :cw], amax[:, :cw],
                                                    xtile[:, :cw], Alu.max)
                            nc.vector.tensor_tensor(amin[:, :cw], amin[:, :cw],
                                                    xtile[:, :cw], Alu.min)
                    nc.vector.tensor_scalar_mul(amin[:, :cw], amin[:, :cw], -1.0)
                    nc.vector.tensor_tensor(amax[:, :cw], amax[:, :cw],
                                            amin[:, :cw], Alu.max)
                    # fold 128 -> 32 partitions
                    ftmp = stage.tile([64, CHUNK], F32, tag="foldx", bufs=1)
                    nc.vector.tensor_copy(ftmp[0:64, :cw], amax[64:128, :cw])
                    nc.vector.tensor_tensor(amax[0:64, :cw], amax[0:64, :cw],
                                            ftmp[0:64, :cw], Alu.max)
                    nc.vector.tensor_copy(ftmp[0:32, :cw], amax[32:64, :cw])
                    nc.vector.tensor_tensor(amax[0:32, :cw], amax[0:32, :cw],
                                            ftmp[0:32, :cw], Alu.max)
                    amt = stage.tile([32, CHUNK], F32, tag="foldx2", bufs=1)
                    nc.vector.transpose(amt[:, :cw], amax[0:32, :cw])
                    red = stage.tile([32, CHUNK // 32], F32, tag="redx", bufs=1)
                    nc.vector.tensor_reduce(
                        red[:], amt[:, :cw].rearrange("p (c q) -> p c q", q=32),
                        axis=mybir.AxisListType.X, op=Alu.max,
                    )
                    nc.vector.tensor_scalar_max(red[:], red[:], EPS)
                    nc.sync.dma_start(
                        sx_ag_in[c0:c0 + cw].rearrange("(c p) -> p c", p=32),
                        red[:],
                    )
                nc.gpsimd.collective_compute(
                    "AllGather", Alu.bypass, replica_groups=rg,
                    ins=[sx_ag_in[:].opt()], outs=[sx_full[:].opt()],
                )

                # ---------- phase Q2: quantize x (transposed layout) ----------
                for c0 in range(0, TS, CHUNK):
                    cw = min(CHUNK, TS - c0)
                    rq_row = stage.tile([1, nb], F32, tag="srow")
                    nc.sync.dma_start(rq_row[:, :cw],
                                      sx_ag_in[c0:c0 + cw].rearrange("(a f) -> a f", a=1))
                    nc.vector.reciprocal(rq_row[:, :cw], rq_row[:, :cw])
                    nc.vector.tensor_scalar_mul(rq_row[:, :cw], rq_row[:, :cw], 127.0)
                    ps = psbc.tile([128, nb], F32, tag="psbc")
                    nc.tensor.matmul(ps[:, :cw], lhsT=ones_row[:], rhs=rq_row[:, :cw],
                                     start=True, stop=True)
                    rq_bcc = stage.tile([128, nb], F32, tag="bc")
                    nc.vector.tensor_copy(rq_bcc[:, :cw], ps[:, :cw])
                    for it in range(KH):
                        xtile = work.tile([128, CHUNK], F32, tag="cf32")
                        nc.sync.dma_start(xtile[:, :cw],
                                          xt_e[it * 128:(it + 1) * 128, c0:c0 + cw])
                        nc.vector.tensor_tensor(xtile[:, :cw], xtile[:, :cw],
                                                rq_bcc[:, :cw], Alu.mult)
                        xqt = work.tile([128, CHUNK], BF16, tag="cbf", bufs=2)
                        nc.vector.tensor_scalar(xqt[:, :cw], xtile[:, :cw],
                                                MAGIC, MAGIC, Alu.add, Alu.subtract)
                        nc.sync.dma_start(
                            xq_ag_in[it * 128:(it + 1) * 128, c0:c0 + cw], xqt[:, :cw]
                        )
                nc.gpsimd.collective_compute(
                    "AllGather", Alu.bypass, replica_groups=rg,
                    ins=[xq_ag_in[:].opt()], outs=[xqT_full[:].opt()],
                )

                if stop_after == "q":
                    return

                # ---------- weight scales from the AllReduce ----------
                sw_sb = singles.tile([1, 2], F32, name="sw_sb")
                nc.sync.dma_start(sw_sb[:], wsum_out[:, :])
                nc.vector.tensor_scalar_mul(sw_sb[:], sw_sb[:], 1.0 / (I * H))
                nc.vector.tensor_scalar_max(sw_sb[:], sw_sb[:], EPS)
                rsw_sb = singles.tile([1, 2], F32, name="rsw_sb")
                nc.vector.reciprocal(rsw_sb[:], sw_sb[:])

                def bcast_scalar(src_ap, name):
                    ps_full = psbc.tile([128, nb], F32, tag="psbc")
                    ps = ps_full[:, 0:1]
                    nc.tensor.matmul(ps, lhsT=ones_row[:], rhs=src_ap,
                                     start=True, stop=True)
                    t = singles.tile([128, 1], F32, name=name)
                    nc.vector.tensor_copy(t[:], ps)
                    return t

                rsw1_col = bcast_scalar(rsw_sb[0:1, 0:1], "rsw1_col")
                rsw2_col = bcast_scalar(rsw_sb[0:1, 1:2], "rsw2_col")
                sw1_127_col = bcast_scalar(sw_sb[0:1, 0:1], "sw1_127_col")
                nc.vector.tensor_scalar_mul(sw1_127_col[:], sw1_127_col[:], 1.0 / 127.0)
                sw2_127_col = bcast_scalar(sw_sb[0:1, 1:2], "sw2_127_col")
                nc.vector.tensor_scalar_mul(sw2_127_col[:], sw2_127_col[:], 1.0 / 127.0)

                def quant_w_tile(dst_ap, src_slice, rsw_col, cols):
                    """Load f32 weight tile, quantize on the fly into dst (bf16)."""
                    for q0 in range(0, cols, CHUNK):
                        qw = min(CHUNK, cols - q0)
                        wt = work.tile([128, CHUNK], F32, tag="cf32")
                        nc.sync.dma_start(wt[:, :qw], src_slice[:, q0:q0 + qw])
                        nc.scalar.mul(wt[:, :qw], wt[:, :qw], rsw_col[:])
                        nc.vector.tensor_scalar(wt[:, :qw], wt[:, :qw], MAGIC, MAGIC,
                                                Alu.add, Alu.subtract)
                        nc.vector.tensor_scalar(dst_ap[:, q0:q0 + qw], wt[:, :qw],
                                                1.0, -1.0, Alu.min, Alu.max)

                # ---------- fc1 ----------
                hred_acc = [singles.tile([32, NT32], F32, name=f"hred_{j}")
                            for j in range(NBLK)]
                for half in range(w1_halves):
                    w1qT = bigw.tile([128, KH, IH], BF16, tag="bigw")
                    for k in range(KH):
                        quant_w_tile(w1qT[:, k, :],
                                     w1t_e[k * 128:(k + 1) * 128,
                                           half * IH:(half + 1) * IH],
                                     rsw1_col, IH)
                    for blk in range(NBLK):
                        crk = blk // BPC           # AG rank chunk
                        coff = (blk % BPC) * nb    # token offset within chunk
                        xq_sb = xqp.tile([128, KH, nb], BF16, tag="xq")
                        for k in range(KH):
                            nc.sync.dma_start(
                                xq_sb[:, k, :],
                                xqT_full[crk * H + k * 128: crk * H + (k + 1) * 128,
                                         coff:coff + nb],
                            )
                        s_row = stage.tile([1, nb], F32, tag="srow")
                        nc.sync.dma_start(
                            s_row[:],
                            sx_full[blk * nb:(blk + 1) * nb].rearrange("(a f) -> a f", a=1),
                        )
                        m1_t = bcast_row(s_row[:], nb)
                        nc.vector.tensor_scalar(m1_t[:], m1_t[:], sw1_127_col[:],
                                                None, Alu.mult)

                        gmax = stage.tile([128, nb], F16, tag="gmax")
                        for m in range(MIH):
                            ps = psum.tile([128, nb], F32, tag="ps1")
                            for k in range(KH):
                                nc.tensor.matmul(
                                    ps[:], lhsT=w1qT[:, k, m * 128:(m + 1) * 128],
                                    rhs=xq_sb[:, k, :],
                                    start=(k == 0), stop=(k == KH - 1),
                                )
                            g = work.tile([128, nb], F32, tag="g", bufs=2)
                            nc.vector.tensor_tensor(g[:], ps[:], m1_t[:], Alu.mult)
                            gq = work.tile([128, nb], F16, tag="gq", bufs=2)
                            nc.scalar.activation(gq[:], g[:], Act.Gelu)
                            nc.sync.dma_start(
                                h_dram[half * IH + m * 128: half * IH + (m + 1) * 128,
                                       blk * nb:(blk + 1) * nb],
                                gq[:],
                            )
                            gabs = work.tile([128, nb], F16, tag="tmph", bufs=2)
                            nc.scalar.activation(gabs[:], gq[:], Act.Abs)
                            if m == 0:
                                nc.vector.tensor_copy(gmax[:], gabs[:])
                            else:
                                nc.vector.tensor_tensor(gmax[:], gmax[:], gabs[:], Alu.max)
                        ftmp = stage.tile([64, nb], F16, tag="foldt", bufs=1)
                        nc.vector.tensor_copy(ftmp[0:64], gmax[64:128])
                        nc.vector.tensor_tensor(gmax[0:64], gmax[0:64], ftmp[0:64], Alu.max)
                        nc.vector.tensor_copy(ftmp[0:32], gmax[32:64])
                        nc.vector.tensor_tensor(gmax[0:32], gmax[0:32], ftmp[0:32], Alu.max)
                        gmt = stage.tile([32, nb], F16, tag="foldt2", bufs=1)
                        nc.vector.transpose(gmt[:], gmax[0:32, :])
                        red = stage.tile([32, NT32], F32, tag="red")
                        nc.vector.tensor_reduce(
                            red[:], gmt[:].rearrange("p (c q) -> p c q", q=32),
                            axis=mybir.AxisListType.X, op=Alu.max,
                        )
                        if half == 0:
                            nc.vector.tensor_copy(hred_acc[blk][:], red[:])
                        else:
                            nc.vector.tensor_tensor(hred_acc[blk][:], hred_acc[blk][:],
                                                    red[:], Alu.max)

                for blk in range(NBLK):
                    nc.sync.dma_start(
                        hmax_in[blk * nb:(blk + 1) * nb].rearrange("(c p) -> p c", p=32),
                        hred_acc[blk][:],
                    )
                nc.gpsimd.collective_compute(
                    "AllReduce", Alu.max, replica_groups=rg,
                    ins=[hmax_in[:].opt()], outs=[hmax_out[:].opt()],
                )

                if stop_after == "fc1":
                    return

                # ---------- fc2 ----------
                # w2 resident in three slots: bigw [KI/2] + xqp [KI/4] x 2
                if KI >= 4:
                    splits = [(0, KI // 2, bigw, "bigw"),
                              (KI // 2, (3 * KI) // 4, xqp, "xq"),
                              ((3 * KI) // 4, KI, xqp, "xq")]
                else:
                    splits = [(0, KI, bigw, "bigw")]
                w2_tiles = []
                for (k0, k1, pool, tag) in splits:
                    wt2 = pool.tile([128, k1 - k0, H], BF16, tag=tag)
                    for ki in range(k0, k1):
                        quant_w_tile(wt2[:, ki - k0, :],
                                     w2t_e[ki * 128:(ki + 1) * 128, :],
                                     rsw2_col, H)
                    w2_tiles.append((k0, k1, wt2))

                def w2_lhsT(ki, msl):
                    for (k0, k1, wt2) in w2_tiles:
                        if k0 <= ki < k1:
                            return wt2[:, ki - k0, msl]
                    raise AssertionError

                for blk in range(NBLK):
                    g_idx = blk // rs_blocks
                    l_off = (blk % rs_blocks) * nb
                    s_row = stage.tile([1, nb], F32, tag="srow")
                    nc.sync.dma_start(
                        s_row[:],
                        hmax_out[blk * nb:(blk + 1) * nb].rearrange("(a f) -> a f", a=1),
                    )
                    nc.vector.tensor_scalar_max(s_row[:], s_row[:], EPS)
                    r_row = stage.tile([1, nb], F32, tag="srow2")
                    nc.vector.reciprocal(r_row[:], s_row[:])
                    rq_t = bcast_row(r_row[:], nb, tag="bcr", bufs=1)
                    nc.vector.tensor_scalar_mul(rq_t[:], rq_t[:], 127.0)
                    m2_t = bcast_row(s_row[:], nb)
                    nc.vector.tensor_scalar(m2_t[:], m2_t[:], sw2_127_col[:],
                                            None, Alu.mult)

                    hq = hqp.tile([128, KI, nb], BF16, tag="hq")
                    for ki in range(KI):
                        hin = work.tile([128, nb], F16, tag="tmph", bufs=2)
                        nc.sync.dma_start(
                            hin[:], h_dram[ki * 128:(ki + 1) * 128,
                                           blk * nb:(blk + 1) * nb]
                        )
                        ht = work.tile([128, nb], F32, tag="g", bufs=2)
                        nc.vector.tensor_tensor(ht[:], hin[:], rq_t[:], Alu.mult)
                        nc.vector.tensor_scalar(hq[:, ki, :], ht[:], MAGIC, MAGIC,
                                                Alu.add, Alu.subtract)
                    for m in range(MH):
                        ps = psum.tile([128, nb], F32, tag="ps1")
                        msl = slice(m * 128, (m + 1) * 128)
                        for ki in range(KI):
                            nc.tensor.matmul(
                                ps[:], lhsT=w2_lhsT(ki, msl), rhs=hq[:, ki, :],
                                start=(ki == 0), stop=(ki == KI - 1),
                            )
                        ot = outp.tile([128, nb], F16, tag="ot")
                        nc.vector.tensor_tensor(ot[:], ps[:], m2_t[:], Alu.mult)
                        nc.sync.dma_start(
                            rs_in[g_idx][m * 128:(m + 1) * 128, l_off:l_off + nb],
                            ot[:],
                        )
                    if blk % rs_blocks == rs_blocks - 1:
                        nc.gpsimd.collective_compute(
                            "ReduceScatter", Alu.add, replica_groups=rg,
                            ins=[rs_in[g_idx][:].opt()], outs=[rs_out[g_idx][:].opt()],
                        )
                        nc.sync.dma_start(
                            out_e[:, g_idx * GT:(g_idx + 1) * GT], rs_out[g_idx][:, :]
                        )

            for rep in range(outer_repeat):
                emit_body(rep)

    nc.compile()
    return nc


_PROGRAM_CACHE = {}


def _get_program(key):
    if key not in _PROGRAM_CACHE:
        _PROGRAM_CACHE[key] = build_program(*key)
    return _PROGRAM_CACHE[key]


def make_in_maps(x, W1, W2, ncores=NCORES):
    t, h = x.reshape(-1, x.shape[-1]).shape
    i = W1.shape[0]
    xf = np.ascontiguousarray(x.reshape(t, h), dtype=np.float32)
    ts, isd = t // ncores, i // ncores
    in_maps = []
    for c in range(ncores):
        xs = xf[c * ts:(c + 1) * ts]
        in_maps.append({
            "xt": np.ascontiguousarray(xs.T),
            "w1t": np.ascontiguousarray(W1[c * isd:(c + 1) * isd, :].T, dtype=np.float32),
            "w2t": np.ascontiguousarray(W2[:, c * isd:(c + 1) * isd].T, dtype=np.float32),
        })
    return in_maps


def run(x, W1, W2, trace=False, trace_kwargs=None):
    """Run the distributed kernel on full inputs. Returns (out, BassKernelResults)."""
    t, h = x.reshape(-1, x.shape[-1]).shape
    i = W1.shape[0]
    nc = _get_program((t, h, i, NCORES))
    in_maps = make_in_maps(x, W1, W2)
    res = bass_utils.run_bass_kernel_spmd(
        nc, in_maps, core_ids=list(range(NCORES)), trace=trace,
        **(trace_kwargs or {}),
    )
    out_t = np.concatenate([res.results[c]["out_t"] for c in range(NCORES)], axis=0)
    out = np.ascontiguousarray(out_t.astype(np.float32).T).reshape(x.shape)
    return out, res


def kernel(x, W1, W2):
    out, _ = run(x, W1, W2)
    return out


class TimedRunner:
    """Compile once, keep inputs on device, time repeated executions.

    Mirrors bass2jax.run_bass_via_pjrt's multi-core path but persists the
    device-side inputs so repeat calls measure (dispatch + HW execution)
    only, not the host->device staging.
    """

    def __init__(self, nc, in_maps):
        import jax
        import concourse.mybir as mybir_
        from concourse import bass2jax
        from jax.experimental.shard_map import shard_map
        from jax.sharding import Mesh, PartitionSpec, NamedSharding

        bass2jax.install_neuronx_cc_hook()
        n_cores = len(in_maps)
        partition_name = nc.partition_id_tensor.name if nc.partition_id_tensor else None
        in_names, out_names, out_avals = [], [], []
        for alloc in nc.m.functions[0].allocations:
            if not isinstance(alloc, mybir_.MemoryLocationSet):
                continue
            name = alloc.memorylocations[0].name
            if alloc.kind == "ExternalInput":
                if name != partition_name:
                    in_names.append(name)
            elif alloc.kind == "ExternalOutput":
                out_names.append(name)
                out_avals.append(jax.core.ShapedArray(
                    tuple(alloc.tensor_shape), mybir_.dt.np(alloc.dtype)))
        n_params = len(in_names)
        n_outs = len(out_avals)
        all_in_names = list(in_names) + list(out_names)
        if partition_name is not None:
            all_in_names.append(partition_name)
        donate = tuple(range(n_params, n_params + n_outs))

        def _body(*args):
            operands = list(args)
            if partition_name is not None:
                operands.append(bass2jax.partition_id_tensor())
            outs = bass2jax._bass_exec_p.bind(
                *operands,
                out_avals=tuple(out_avals),
                in_names=tuple(all_in_names),
                out_names=tuple(out_names),
                lowering_input_output_aliases=(),
                sim_require_finite=True,
                sim_require_nnan=True,
                nc=nc,
            )
            return tuple(outs)

        devices = jax.devices()[:n_cores]
        mesh = Mesh(np.asarray(devices), ("core",))
        in_specs = (PartitionSpec("core"),) * (n_params + n_outs)
        out_specs = (PartitionSpec("core"),) * n_outs
        self._fn = jax.jit(
            shard_map(_body, mesh=mesh, in_specs=in_specs, out_specs=out_specs,
                      check_rep=False),
            donate_argnums=donate, keep_unused=True,
        )
        sh = NamedSharding(mesh, PartitionSpec("core"))
        concat_in = [
            np.concatenate([np.asarray(in_maps[c][nm]) for c in range(n_cores)], axis=0)
            for nm in in_names
        ]
        self._dev_in = [jax.device_put(a, sh) for a in concat_in]
        self._zero_shapes = [(n_cores * a.shape[0], *a.shape[1:]) for a in out_avals]
        self._zero_dtypes = [a.dtype for a in out_avals]
        self._sh = sh
        self._jax = jax
        self.out_names = out_names
        self.out_avals = out_avals
        self.n_cores = n_cores

    def run_once(self):
        import time
        jax = self._jax
        zeros = [jax.device_put(np.zeros(s, d), self._sh)
                 for s, d in zip(self._zero_shapes, self._zero_dtypes)]
        jax.block_until_ready(zeros)
        t0 = time.perf_counter()
        outs = self._fn(*self._dev_in, *zeros)
        jax.block_until_ready(outs)
        dt = time.perf_counter() - t0
        results = [
            {nm: np.asarray(outs[i]).reshape(self.n_cores, *self.out_avals[i].shape)[c]
             for i, nm in enumerate(self.out_names)}
            for c in range(self.n_cores)
        ]
        return results, dt
